# revision 17
# baseline (speedup 1.0000x reference)
"""Multi-head self-attention Bass/Tile kernel for Trainium2, 8 NeuronCores.

Problem: B=4, S=2048, D=1024, H=16 heads (HD=64), fp32, causal mask,
no padding.  y = softmax((xWq+bq)(xWk+bk)^T / 8 + mask) (xWv+bv) Wo + bo

Sharding (4-way batch x 2-way head-group):
  core c -> batch b = c//2, head group g = c%2 (heads 8g..8g+7).
  Each core computes its 8 heads' attention output and a PARTIAL
  out-projection y_partial = attn_out @ Wout[rows of its heads] (+ bout
  on g==0 cores only).  Host sums the two partials per batch.

Design (bf16 matmuls + fp8 Q/K projection, f32 PSUM accumulation):
  Parity trick: K^T stays as natural head-PAIRS [128, S] in SBUF (rows
  0-63 = even head, 64-127 = odd head, never split or zero-padded).
  Q^T is stored per head [128, S] with the OPPOSITE 64 partitions ZERO,
  so the full-128-deep scores matmul K_pair^T.T @ Q_h contracts to
  exactly one head's scores.  Everything stays SBUF-resident.
  Q/K projections run in fp8 e4m3 with DoubleRow perf mode (256-deep
  contraction at 0.5 cyc/row); the resulting ~3% q/k noise only enters
  the softmax scores, where attention averaging launders it (~1.4e-2
  final rel err vs the 2e-2 gate).  V and the output path stay bf16.
  Exp batching: scores for groups of 2 k-chunks land in one 2-bank
  PSUM tile; one ACT exp covers 1024 columns, amortizing the ~350-cycle
  ACT fixed overhead.  AV matmuls are emitted two exp-groups late
  (software pipelining across unit boundaries) so the in-order PE queue
  always has scores work while ACT evaluates exps; B1 runs qt-outer
  and B3 (normalize) + C (out-proj) for q-tile qt are emitted after
  B1(qt+1), hiding the reciprocal chain behind PE work.
  Causal masking: only the [128,128] diagonal triangle gets a DVE
  mask-add; fully-masked columns of diagonal blocks are skipped by
  accumulating the AV matmul over a column sub-range.
  Softmax denominator: one-hot column 64+h of V_aug makes the AV
  matmul accumulate head h's denominator on PSUM row 64+h for free.
  DMA discipline: the shared DMA engine chokes on small/broadcast
  descriptors (they starved the weight loads for ~150us in v2), so all
  transfers are few and fat from host-prearranged layouts; one-hots
  and padding zeros are built on-chip with memsets; V/out-proj biases
  ride the PSUM accumulation as rank-1 matmuls (ones x bias_row).
  Known pitfalls hit on this stack: reciprocal_approx_fast silently
  no-ops; f32r operands in the B3 broadcast matmul are rejected by NRT
  at load time (use bf16 staging); engine PSUM access needs 32/64-
  aligned partition offsets.
"""

import sys

if "/opt/trn_rl_repo" not in sys.path:
    sys.path.insert(0, "/opt/trn_rl_repo")

import ml_dtypes
import numpy as np

import concourse.bass as bass
import concourse.mybir as mybir
import concourse.tile as tile
from concourse import bacc
from concourse.bass_utils import run_bass_kernel_spmd

f32 = mybir.dt.float32
BF16 = mybir.dt.bfloat16
F32R = mybir.dt.float32r
FP8 = mybir.dt.float8e4
DR = mybir.MatmulPerfMode.DoubleRow
AF = mybir.ActivationFunctionType
OP = mybir.AluOpType

B, S, D, H = 4, 2048, 1024, 16
HD = D // H            # 64
P = 128
DC = D // P            # 8 contraction chunks for the projections
NPAIR = 4              # head pairs per core (8 local heads)
NST = S // 512         # 4 S-tiles of 512
NKC = S // P           # 16 k-chunks of 128
VW = HD + 8            # V_aug width: 64 V cols + 8 one-hot denominator cols
NEG = -1.0e30
EG = 2                 # k-chunks per exp group (2 PSUM banks)


def build_program():
    nc = bacc.Bacc("TRN2", target_bir_lowering=False, debug=False)

    # host-prearranged layouts so every DMA is a fat contiguous transfer
    xt_d = nc.dram_tensor("xt", [P, NST, DC, 512], BF16, kind="ExternalInput")
    wv_d = nc.dram_tensor("wv", [P, DC, 512], BF16, kind="ExternalInput")
    w8_d = nc.dram_tensor("w8qk", [2, P, 4, 2, 512], FP8, kind="ExternalInput")
    xt8_d = nc.dram_tensor("xt8", [P, NST, 4, 2, 512], FP8, kind="ExternalInput")
    bq_d = nc.dram_tensor("bq12", [P, 12], f32, kind="ExternalInput")
    bv_d = nc.dram_tensor("bv", [1, 512], BF16, kind="ExternalInput")
    bo_d = nc.dram_tensor("bo", [1, D], BF16, kind="ExternalInput")
    wo_d = nc.dram_tensor("wout", [P, 4, D], BF16, kind="ExternalInput")
    cm_d = nc.dram_tensor("cmtri", [P, P], BF16, kind="ExternalInput")
    sel_d = nc.dram_tensor("sel", [8, 8, HD], BF16, kind="ExternalInput")
    y_d = nc.dram_tensor("y", [S, D], f32, kind="ExternalOutput")

    from contextlib import ExitStack

    with tile.TileContext(nc) as tc, ExitStack() as _lp:
        _lp.enter_context(
            nc.allow_low_precision(reason="bf16 matmuls with f32 psum accumulation")
        )
        with tc.tile_pool(name="pers", bufs=1) as pers, \
             tc.tile_pool(name="consts", bufs=1) as consts:

            # ---- persistent activations ----
            # Q^T per head: live 64 rows at parity offset, other 64 rows ZERO
            q_all = pers.tile([P, 8, S], BF16, tag="q")
            # K^T natural head pairs (rows 0-63 even head, 64-127 odd head)
            kt_all = pers.tile([P, NPAIR, S], BF16, tag="kt")
            v_all = pers.tile([P, NKC, 8, VW], BF16, tag="v")
            attn_t = pers.tile([P, NPAIR, S], BF16, tag="attn")
            den = pers.tile([P, NST, 512], f32, tag="den")

            # ---- constants ----
            sel_sb = consts.tile([72, 8, HD], BF16, tag="sel")
            bq_sb = consts.tile([P, 12], f32, tag="bq")
            bv_sb = consts.tile([1, 512], BF16, tag="bv")
            bo_sb = consts.tile([1, D], BF16, tag="bo")
            cm_sb = consts.tile([P, P], BF16, tag="cm")
            wo_sb = consts.tile([P, 4, D], BF16, tag="wout")
            ones = consts.tile([1, P], BF16, tag="ones")

            # ---- on-chip init (no DMA): zeros/one-hots via DVE memsets ----
            nc.vector.memset(ones[0:1, :], 1.0)
            for h in range(8):
                dead = slice(HD, P) if h % 2 == 0 else slice(0, HD)
                nc.vector.memset(q_all[dead, h, :], 0.0)
            # one-hot denominator columns of V_aug: col 64+j = (j == h)
            nc.vector.memset(v_all[:, :, :, HD:VW], 0.0)
            for h in range(8):
                nc.vector.memset(v_all[:, :, h, HD + h : HD + h + 1], 1.0)

            # ================= Stage A: QKV projections =================
            with tc.tile_pool(name="wqkvp", bufs=1) as wqkvp, \
                 tc.tile_pool(name="xtp", bufs=1) as xtp, \
                 tc.tile_pool(name="ps_mm", bufs=6, space="PSUM") as ps_mm:

                wv_sb = wqkvp.tile([P, DC, 512], BF16, tag="wv")
                w8_sb = wqkvp.tile([P, 2, 4, 2, 512], FP8, tag="w8qk")
                xt = xtp.tile([P, NST, DC, 512], BF16, tag="xt")
                xt8 = xtp.tile([P, NST, 4, 2, 512], FP8, tag="xt8")

                # big inputs on the Sync (HWDGE) queue, interleaved so the
                # first-needed chunk lands first; consts on the GpSimd queue
                nc.sync.dma_start(out=w8_sb[:], in_=w8_d.rearrange("k p a b c -> p k a b c"))
                nc.sync.dma_start(out=xt8[:, 0], in_=xt8_d[:, 0])
                nc.sync.dma_start(out=xt[:, 0], in_=xt_d[:, 0])
                nc.sync.dma_start(out=wv_sb[:], in_=wv_d[:])
                for st in range(1, NST):
                    nc.sync.dma_start(out=xt8[:, st], in_=xt8_d[:, st])
                    nc.sync.dma_start(out=xt[:, st], in_=xt_d[:, st])
                nc.gpsimd.dma_start(out=bq_sb[:], in_=bq_d[:])
                nc.gpsimd.dma_start(out=cm_sb[:], in_=cm_d[:])
                nc.gpsimd.dma_start(out=sel_sb[64:72, :, :], in_=sel_d[:])
                nc.gpsimd.dma_start(out=bv_sb[:], in_=bv_d[:])
                nc.gpsimd.dma_start(out=bo_sb[:], in_=bo_d[:])
                nc.gpsimd.dma_start(out=wo_sb[:], in_=wo_d[:])

                for st in range(NST):
                    sl = slice(st * 512, (st + 1) * 512)
                    # Q^T head-pair tiles -> parity-split per-head SBUF layout
                    for pr in range(NPAIR):
                        mm = ps_mm.tile([P, 512], f32, tag="mm")
                        for j in range(4):
                            nc.tensor.matmul(
                                mm[:],
                                w8_sb[:, 0, j, :, pr * P : (pr + 1) * P],
                                xt8[:, st, j, :, :],
                                start=(j == 0),
                                stop=(j == 3),
                                perf_mode=DR,
                            )
                        bcol = bq_sb[:, pr : pr + 1]
                        # even head: live rows 0-63; odd head: live rows 64-127
                        nc.scalar.activation(
                            out=q_all[0:HD, 2 * pr, sl],
                            in_=mm[0:HD, :],
                            func=AF.Identity,
                            bias=bcol[0:HD],
                        )
                        nc.scalar.activation(
                            out=q_all[HD:P, 2 * pr + 1, sl],
                            in_=mm[HD:P, :],
                            func=AF.Identity,
                            bias=bcol[HD:P],
                        )
                    # K^T head-pair tiles -> resident pair-packed SBUF layout
                    for pr in range(NPAIR):
                        mm = ps_mm.tile([P, 512], f32, tag="mm")
                        for j in range(4):
                            nc.tensor.matmul(
                                mm[:],
                                w8_sb[:, 1, j, :, pr * P : (pr + 1) * P],
                                xt8[:, st, j, :, :],
                                start=(j == 0),
                                stop=(j == 3),
                                perf_mode=DR,
                            )
                        nc.scalar.activation(
                            out=kt_all[:, pr, sl],
                            in_=mm[:],
                            func=AF.Identity,
                            bias=bq_sb[:, 4 + pr : 5 + pr],
                        )
                    # V: natural [S, hd] layout per 128-row chunk, all 8 heads;
                    # bias rides the accumulation as a rank-1 matmul
                    for sb in range(4):
                        mm = ps_mm.tile([P, 512], f32, tag="mm")
                        for dc in range(DC):
                            nc.tensor.matmul(
                                mm[:],
                                xt[:, st, dc, sb * P : (sb + 1) * P],
                                wv_sb[:, dc, :],
                                start=(dc == 0),
                                stop=False,
                            )
                        nc.tensor.matmul(
                            mm[:],
                            ones[0:1, :],
                            bv_sb[0:1, :],
                            start=False,
                            stop=True,
                        )
                        kc = st * 4 + sb
                        nc.vector.tensor_copy(
                            out=v_all[:, kc, :, 0:HD],
                            in_=mm[:].rearrange("p (h d) -> p h d", h=8),
                        )

            # ================= Stage B: attention =================
            with tc.tile_pool(name="ppool", bufs=3) as ppool, \
                 tc.tile_pool(name="ystage", bufs=3) as ystage, \
                 tc.tile_pool(name="rbp", bufs=2) as rbp, \
                 tc.tile_pool(name="ps_sg", bufs=3, space="PSUM") as ps_sg, \
                 tc.tile_pool(name="ps_av", bufs=2, space="PSUM") as ps_av:
                # B1 runs qt-outer so each q-tile's denominators finish early;
                # B3+C for q-tile qt are emitted after B1(qt+1)'s units, which
                # keeps the in-order PE queue busy while the reciprocal chain
                # runs on the DVE.  AV matmuls are emitted two exp-groups late
                # (software pipelining across unit boundaries) so the PE
                # always has scores work while ACT evaluates exps.
                from collections import deque
                pending = deque()

                def flush_one():
                    if pending:
                        pending.popleft()()

                def emit_unit(h, qt):
                    pr, half = h // 2, h % 2
                    q0 = qt * 512
                    nk = 4 * qt + 4
                    av = ps_av.tile([P, 512], f32, tag="avy", name=f"av{h}_{qt}")
                    for g0 in range(0, nk, EG):
                        gsz = min(EG, nk - g0)
                        sg = ps_sg.tile([P, EG * 512], f32, tag="sg",
                                        name=f"sg{h}_{qt}_{g0}")
                        for j in range(gsz):
                            kc = g0 + j
                            js = slice(j * 512, (j + 1) * 512)
                            nc.tensor.matmul(
                                sg[:, js],
                                kt_all[:, pr, kc * P : (kc + 1) * P],
                                q_all[:, h, q0 : q0 + 512],
                                start=True,
                                stop=True,
                            )
                            m = kc - 4 * qt
                            if m >= 0:
                                # triangular mask on the [128,128] diagonal
                                nc.vector.tensor_tensor(
                                    sg[:, j * 512 + m * P : j * 512 + (m + 1) * P],
                                    sg[:, j * 512 + m * P : j * 512 + (m + 1) * P],
                                    cm_sb[:],
                                    OP.add,
                                )
                        pt = ppool.tile([P, EG * 512], BF16, tag="pt",
                                        name=f"pt{h}_{qt}_{g0}")
                        nc.scalar.activation(
                            out=pt[:, 0 : gsz * 512],
                            in_=sg[:, 0 : gsz * 512],
                            func=AF.Exp,
                            scale=0.125,
                        )
                        if len(pending) >= 2:
                            flush_one()

                        def av_group(h=h, qt=qt, g0=g0, gsz=gsz, pt=pt, av=av,
                                     last=(g0 + gsz == nk)):
                            nk_ = 4 * qt + 4
                            for j in range(gsz):
                                kc = g0 + j
                                m = kc - 4 * qt
                                c0 = m * P if m > 0 else 0
                                nc.tensor.matmul(
                                    av[0:VW, c0:512],
                                    v_all[:, kc, h, :],
                                    pt[:, j * 512 + c0 : (j + 1) * 512],
                                    start=(kc == 0),
                                    stop=(kc == nk_ - 1),
                                    skip_group_check=True,
                                )
                            if not last:
                                return
                            # park unnormalized output + denominator (row
                            # 64+h of av holds head h's denominator, other
                            # rows are zero, so accumulating the aligned
                            # [64:72] block is exact)
                            if h == 0:
                                nc.vector.tensor_copy(
                                    out=den[64:72, qt, :],
                                    in_=av[64:72, :],
                                )
                            else:
                                nc.vector.tensor_tensor(
                                    den[64:72, qt, :],
                                    den[64:72, qt, :],
                                    av[64:72, :],
                                    OP.add,
                                )
                            po_ = HD * (h % 2)
                            nc.vector.tensor_copy(
                                out=attn_t[po_ : po_ + HD, h // 2,
                                           qt * 512 : qt * 512 + 512],
                                in_=av[0:HD, :],
                            )

                        pending.append(av_group)

                def emit_recip(qt):
                    nc.vector.reciprocal(den[64:72, qt, :], den[64:72, qt, :])

                def emit_b3(qt):
                    q0 = qt * 512
                    denb = rbp.tile([72, 512], BF16, tag="denb", name=f"denb{qt}")
                    nc.scalar.activation(
                        out=denb[64:72, :],
                        in_=den[64:72, qt, :],
                        func=AF.Identity,
                    )
                    for h in range(8):
                        pr, half = h // 2, h % 2
                        po = HD * half
                        rb = ps_av.tile([P, 512], f32, tag="avy",
                                        name=f"rb{qt}_{h}")
                        nc.tensor.matmul(
                            rb[0:HD, :],
                            sel_sb[64:72, h, :],
                            denb[64:72, :],
                            start=True,
                            stop=True,
                        )
                        nc.vector.tensor_tensor(
                            attn_t[po : po + HD, pr, q0 : q0 + 512],
                            attn_t[po : po + HD, pr, q0 : q0 + 512],
                            rb[0:HD, :],
                            OP.mult,
                        )

                def emit_c(qt, half=None):
                    qcs = range(4 * qt, 4 * qt + 4)
                    if half == 0:
                        qcs = range(4 * qt, 4 * qt + 2)
                    elif half == 1:
                        qcs = range(4 * qt + 2, 4 * qt + 4)
                    for qc in qcs:
                        q0 = qc * P
                        yt = ystage.tile([P, D], f32, tag="yt", name=f"yt{qc}")
                        for nb in range(2):
                            # yp lives in the 3-deep sg pool: its rotation
                            # partners drain via fast ACT exps, unlike the
                            # avy tag whose rb tiles wait on DVE mults
                            yp = ps_sg.tile([P, EG * 512], f32, tag="sg",
                                            name=f"yp{qc}_{nb}")
                            for pc in range(4):
                                nc.tensor.matmul(
                                    yp[:, 0:512],
                                    attn_t[:, pc, q0 : q0 + P],
                                    wo_sb[:, pc, nb * 512 : (nb + 1) * 512],
                                    start=(pc == 0),
                                    stop=False,
                                )
                            nc.tensor.matmul(
                                yp[:, 0:512],
                                ones[0:1, :],
                                bo_sb[0:1, nb * 512 : (nb + 1) * 512],
                                start=False,
                                stop=True,
                            )
                            nc.vector.tensor_copy(
                                out=yt[:, nb * 512 : (nb + 1) * 512],
                                in_=yp[:, 0:512],
                            )
                        nc.sync.dma_start(out=y_d[q0 : q0 + P, :], in_=yt[:])

                # B3/C for q-tile qt-1 are spread INSIDE round qt so the
                # reciprocal chain always has PE work as cover and the kernel
                # tail holds only B3(3) + C(3); half of C(2) is held back to
                # cover the final reciprocal chain.
                for qt in range(NST):
                    for h in range(8):
                        emit_unit(h, qt)
                        if qt >= 1:
                            if h == 1:
                                emit_recip(qt - 1)
                            elif h == 3:
                                emit_b3(qt - 1)
                            elif h == 5:
                                emit_c(qt - 1, half=0)
                            elif h == 7 and qt < NST - 1:
                                emit_c(qt - 1, half=1)
                while pending:
                    flush_one()
                emit_recip(NST - 1)
                emit_c(NST - 2, half=1)
                emit_b3(NST - 1)
                emit_c(NST - 1)

    nc.finalize()
    return nc


_NC = None


def _get_nc():
    global _NC
    if _NC is None:
        _NC = build_program()
    return _NC


def _shard_inputs(x, causal_mask, padding_mask, W_qkv, b_qkv, W_out, b_out):
    bf16 = ml_dtypes.bfloat16
    x = np.ascontiguousarray(np.asarray(x, dtype=np.float32))
    W_qkv = np.asarray(W_qkv, dtype=np.float32)
    b_qkv = np.asarray(b_qkv, dtype=np.float32)
    W_out = np.asarray(W_out, dtype=np.float32)
    b_out = np.asarray(b_out, dtype=np.float32)
    causal_mask = np.asarray(causal_mask)
    padding_mask = np.asarray(padding_mask)

    assert not padding_mask.any(), "kernel assumes no padding"
    # additive triangle for the [128,128] diagonal block of scores^T[k, q]:
    # masked iff local k > local q
    cm = np.where(
        causal_mask[0:P, 0:P].T, np.float32(NEG), np.float32(0.0)
    ).astype(bf16)
    sel = np.repeat(np.eye(8, dtype=np.float32)[:, :, None], HD, axis=2).astype(bf16)

    in_maps = []
    for c in range(8):
        b, g = c // 2, c % 2
        cols = slice(g * 512, (g + 1) * 512)
        # [3, 1024, 512]: per-projection weight slices for this head group
        w3 = np.stack(
            [W_qkv[:, 1024 * i : 1024 * (i + 1)][:, cols] for i in range(3)]
        )
        wv = np.ascontiguousarray(
            w3[2].reshape(DC, P, 512).transpose(1, 0, 2).astype(bf16)
        )
        fp8 = ml_dtypes.float8_e4m3
        w8 = np.ascontiguousarray(
            w3[0:2].reshape(2, 4, 2, P, 512).transpose(0, 3, 1, 2, 4).astype(fp8)
        )
        xt8 = np.ascontiguousarray(
            x[b].T.reshape(4, 2, P, NST, 512).transpose(2, 3, 0, 1, 4).astype(fp8)
        )
        b3 = np.stack([b_qkv[1024 * i : 1024 * (i + 1)][cols] for i in range(3)])
        bq12 = np.ascontiguousarray(b3[0:2].reshape(8, P).T.astype(np.float32))
        bq12 = np.concatenate(
            [bq12, np.zeros((P, 4), np.float32)], axis=1
        )  # [128, 12]; V-bias columns unused
        xt = np.ascontiguousarray(
            x[b].T.reshape(DC, P, NST, 512).transpose(1, 2, 0, 3).astype(bf16)
        )
        wo = np.ascontiguousarray(
            W_out[g * 512 : (g + 1) * 512, :]
            .reshape(4, P, D)
            .transpose(1, 0, 2)
            .astype(bf16)
        )
        in_maps.append(
            {
                "xt": xt,
                "wv": wv,
                "w8qk": w8,
                "xt8": xt8,
                "bq12": bq12,
                "bv": np.ascontiguousarray(b3[2:3].astype(bf16)),
                "bo": (b_out if g == 0 else np.zeros_like(b_out))[None, :].astype(bf16),
                "wout": wo,
                "cmtri": cm,
                "sel": sel,
            }
        )
    return in_maps


def _run(in_maps, **kwargs):
    nc = _get_nc()
    return run_bass_kernel_spmd(nc, in_maps, core_ids=list(range(8)), **kwargs)


def kernel(**inputs):
    in_maps = _shard_inputs(**inputs)
    res = _run(in_maps)
    out = np.empty((B, S, D), dtype=np.float32)
    for b in range(B):
        out[b] = res.results[2 * b]["y"] + res.results[2 * b + 1]["y"]
    return out


def kernel_traced(**inputs):
    """Like kernel() but with NTFF tracing; returns (out, BassKernelResults)."""
    in_maps = _shard_inputs(**inputs)
    res = _run(in_maps, trace=True)
    out = np.empty((B, S, D), dtype=np.float32)
    for b in range(B):
        out[b] = res.results[2 * b]["y"] + res.results[2 * b + 1]["y"]
    return out, res


# revision 19
# speedup vs baseline: 1.0237x; 1.0237x over previous
"""Multi-head self-attention Bass/Tile kernel for Trainium2, 8 NeuronCores.

Problem: B=4, S=2048, D=1024, H=16 heads (HD=64), fp32, causal mask,
no padding.  y = softmax((xWq+bq)(xWk+bk)^T / 8 + mask) (xWv+bv) Wo + bo

Sharding (4-way batch x 2-way head-group):
  core c -> batch b = c//2, head group g = c%2 (heads 8g..8g+7).
  Each core computes its 8 heads' attention output and a PARTIAL
  out-projection y_partial = attn_out @ Wout[rows of its heads] (+ bout
  on g==0 cores only).  Host sums the two partials per batch.

Design (bf16 matmuls + fp8 Q/K projection, f32 PSUM accumulation):
  Parity trick: K^T stays as natural head-PAIRS [128, S] in SBUF (rows
  0-63 = even head, 64-127 = odd head, never split or zero-padded).
  Q^T is stored per head [128, S] with the OPPOSITE 64 partitions ZERO,
  so the full-128-deep scores matmul K_pair^T.T @ Q_h contracts to
  exactly one head's scores.  Everything stays SBUF-resident.
  Q/K projections run in fp8 e4m3 with DoubleRow perf mode (256-deep
  contraction at 0.5 cyc/row); the resulting ~3% q/k noise only enters
  the softmax scores, where attention averaging launders it (~1.4e-2
  final rel err vs the 2e-2 gate).  V and the output path stay bf16.
  Exp batching: scores for groups of 2 k-chunks land in one 2-bank
  PSUM tile; one ACT exp covers 1024 columns, amortizing the ~350-cycle
  ACT fixed overhead.  AV matmuls are emitted two exp-groups late
  (software pipelining across unit boundaries) so the in-order PE queue
  always has scores work while ACT evaluates exps; B1 runs qt-outer
  and B3 (normalize) + C (out-proj) for q-tile qt are emitted after
  B1(qt+1), hiding the reciprocal chain behind PE work.
  Causal masking: only the [128,128] diagonal triangle gets a DVE
  mask-add; fully-masked columns of diagonal blocks are skipped by
  accumulating the AV matmul over a column sub-range.
  Softmax denominator: one-hot column 64+h of V_aug makes the AV
  matmul accumulate head h's denominator on PSUM row 64+h for free.
  DMA discipline: the shared DMA engine chokes on small/broadcast
  descriptors (they starved the weight loads for ~150us in v2), so all
  transfers are few and fat from host-prearranged layouts; one-hots
  and padding zeros are built on-chip with memsets; V/out-proj biases
  ride the PSUM accumulation as rank-1 matmuls (ones x bias_row).
  Known pitfalls hit on this stack: reciprocal_approx_fast silently
  no-ops; f32r operands in the B3 broadcast matmul are rejected by NRT
  at load time (use bf16 staging); engine PSUM access needs 32/64-
  aligned partition offsets.
"""

import sys

if "/opt/trn_rl_repo" not in sys.path:
    sys.path.insert(0, "/opt/trn_rl_repo")

import ml_dtypes
import numpy as np

import concourse.bass as bass
import concourse.mybir as mybir
import concourse.tile as tile
from concourse import bacc
from concourse.bass_utils import run_bass_kernel_spmd

f32 = mybir.dt.float32
BF16 = mybir.dt.bfloat16
F32R = mybir.dt.float32r
FP8 = mybir.dt.float8e4
DR = mybir.MatmulPerfMode.DoubleRow
AF = mybir.ActivationFunctionType
OP = mybir.AluOpType

B, S, D, H = 4, 2048, 1024, 16
HD = D // H            # 64
P = 128
DC = D // P            # 8 contraction chunks for the projections
NPAIR = 4              # head pairs per core (8 local heads)
NST = S // 512         # 4 S-tiles of 512
NKC = S // P           # 16 k-chunks of 128
VW = HD + 8            # V_aug width: 64 V cols + 8 one-hot denominator cols
NEG = -1.0e30
EG = 2                 # k-chunks per exp group (2 PSUM banks)


def build_program():
    nc = bacc.Bacc("TRN2", target_bir_lowering=False, debug=False)

    # host-prearranged layouts so every DMA is a fat contiguous transfer
    xt_d = nc.dram_tensor("xt", [P, NST, DC, 512], BF16, kind="ExternalInput")
    wv_d = nc.dram_tensor("wv", [P, DC, 512], BF16, kind="ExternalInput")
    w8_d = nc.dram_tensor("w8qk", [2, P, 4, 2, 512], FP8, kind="ExternalInput")
    xt8_d = nc.dram_tensor("xt8", [P, NST, 4, 2, 512], FP8, kind="ExternalInput")
    bq_d = nc.dram_tensor("bq12", [P, 12], f32, kind="ExternalInput")
    bv_d = nc.dram_tensor("bv", [1, 512], BF16, kind="ExternalInput")
    bo_d = nc.dram_tensor("bo", [1, D], BF16, kind="ExternalInput")
    wo_d = nc.dram_tensor("wout", [P, 4, D], BF16, kind="ExternalInput")
    cm_d = nc.dram_tensor("cmtri", [P, P], BF16, kind="ExternalInput")
    sel_d = nc.dram_tensor("sel", [8, 8, HD], BF16, kind="ExternalInput")
    y_d = nc.dram_tensor("y", [S, D], f32, kind="ExternalOutput")

    from contextlib import ExitStack

    with tile.TileContext(nc) as tc, ExitStack() as _lp:
        _lp.enter_context(
            nc.allow_low_precision(reason="bf16 matmuls with f32 psum accumulation")
        )
        with tc.tile_pool(name="pers", bufs=1) as pers, \
             tc.tile_pool(name="consts", bufs=1) as consts:

            # ---- persistent activations ----
            # Q^T per head: live 64 rows at parity offset, other 64 rows ZERO
            q_all = pers.tile([P, 8, S], BF16, tag="q")
            # K^T natural head pairs (rows 0-63 even head, 64-127 odd head)
            kt_all = pers.tile([P, NPAIR, S], BF16, tag="kt")
            v_all = pers.tile([P, NKC, 8, VW], BF16, tag="v")
            attn_t = pers.tile([P, NPAIR, S], BF16, tag="attn")
            den = pers.tile([P, NST, 512], f32, tag="den")

            # ---- constants ----
            sel_sb = consts.tile([72, 8, HD], BF16, tag="sel")
            bq_sb = consts.tile([P, 12], f32, tag="bq")
            bv_sb = consts.tile([1, 512], BF16, tag="bv")
            bo_sb = consts.tile([1, D], BF16, tag="bo")
            cm_sb = consts.tile([P, P], BF16, tag="cm")
            wo_sb = consts.tile([P, 4, D], BF16, tag="wout")
            ones = consts.tile([1, P], BF16, tag="ones")

            # ---- on-chip init (no DMA): zeros/one-hots via DVE memsets ----
            nc.vector.memset(ones[0:1, :], 1.0)
            for h in range(8):
                dead = slice(HD, P) if h % 2 == 0 else slice(0, HD)
                nc.vector.memset(q_all[dead, h, :], 0.0)
            # one-hot denominator columns of V_aug: col 64+j = (j == h)
            nc.vector.memset(v_all[:, :, :, HD:VW], 0.0)
            for h in range(8):
                nc.vector.memset(v_all[:, :, h, HD + h : HD + h + 1], 1.0)

            # ================= Stage A: QKV projections =================
            with tc.tile_pool(name="wqkvp", bufs=1) as wqkvp, \
                 tc.tile_pool(name="xtp", bufs=1) as xtp, \
                 tc.tile_pool(name="ps_mm", bufs=6, space="PSUM") as ps_mm:

                wv_sb = wqkvp.tile([P, DC, 512], BF16, tag="wv")
                w8_sb = wqkvp.tile([P, 2, 4, 2, 512], FP8, tag="w8qk")
                xt = xtp.tile([P, NST, DC, 512], BF16, tag="xt")
                xt8 = xtp.tile([P, NST, 4, 2, 512], FP8, tag="xt8")

                # big inputs on the Sync (HWDGE) queue, interleaved so the
                # first-needed chunk lands first; consts on the GpSimd queue
                nc.sync.dma_start(out=w8_sb[:], in_=w8_d.rearrange("k p a b c -> p k a b c"))
                nc.sync.dma_start(out=xt8[:, 0], in_=xt8_d[:, 0])
                nc.sync.dma_start(out=xt[:, 0], in_=xt_d[:, 0])
                nc.sync.dma_start(out=wv_sb[:], in_=wv_d[:])
                for st in range(1, NST):
                    nc.sync.dma_start(out=xt8[:, st], in_=xt8_d[:, st])
                    nc.sync.dma_start(out=xt[:, st], in_=xt_d[:, st])
                nc.gpsimd.dma_start(out=bq_sb[:], in_=bq_d[:])
                nc.gpsimd.dma_start(out=cm_sb[:], in_=cm_d[:])
                nc.gpsimd.dma_start(out=sel_sb[64:72, :, :], in_=sel_d[:])
                nc.gpsimd.dma_start(out=bv_sb[:], in_=bv_d[:])
                nc.gpsimd.dma_start(out=bo_sb[:], in_=bo_d[:])
                nc.gpsimd.dma_start(out=wo_sb[:], in_=wo_d[:])

                for st in range(NST):
                    sl = slice(st * 512, (st + 1) * 512)
                    # Q^T head-pair tiles -> parity-split per-head SBUF layout
                    for pr in range(NPAIR):
                        mm = ps_mm.tile([P, 512], f32, tag="mm")
                        for j in range(4):
                            nc.tensor.matmul(
                                mm[:],
                                w8_sb[:, 0, j, :, pr * P : (pr + 1) * P],
                                xt8[:, st, j, :, :],
                                start=(j == 0),
                                stop=(j == 3),
                                perf_mode=DR,
                            )
                        bcol = bq_sb[:, pr : pr + 1]
                        # even head: live rows 0-63; odd head: live rows 64-127
                        nc.scalar.activation(
                            out=q_all[0:HD, 2 * pr, sl],
                            in_=mm[0:HD, :],
                            func=AF.Identity,
                            bias=bcol[0:HD],
                        )
                        nc.scalar.activation(
                            out=q_all[HD:P, 2 * pr + 1, sl],
                            in_=mm[HD:P, :],
                            func=AF.Identity,
                            bias=bcol[HD:P],
                        )
                    # K^T head-pair tiles -> resident pair-packed SBUF layout
                    for pr in range(NPAIR):
                        mm = ps_mm.tile([P, 512], f32, tag="mm")
                        for j in range(4):
                            nc.tensor.matmul(
                                mm[:],
                                w8_sb[:, 1, j, :, pr * P : (pr + 1) * P],
                                xt8[:, st, j, :, :],
                                start=(j == 0),
                                stop=(j == 3),
                                perf_mode=DR,
                            )
                        nc.scalar.activation(
                            out=kt_all[:, pr, sl],
                            in_=mm[:],
                            func=AF.Identity,
                            bias=bq_sb[:, 4 + pr : 5 + pr],
                        )
                    # V: natural [S, hd] layout per 128-row chunk, all 8 heads;
                    # bias rides the accumulation as a rank-1 matmul
                    for sb in range(4):
                        mm = ps_mm.tile([P, 512], f32, tag="mm")
                        for dc in range(DC):
                            nc.tensor.matmul(
                                mm[:],
                                xt[:, st, dc, sb * P : (sb + 1) * P],
                                wv_sb[:, dc, :],
                                start=(dc == 0),
                                stop=False,
                            )
                        nc.tensor.matmul(
                            mm[:],
                            ones[0:1, :],
                            bv_sb[0:1, :],
                            start=False,
                            stop=True,
                        )
                        kc = st * 4 + sb
                        nc.vector.tensor_copy(
                            out=v_all[:, kc, :, 0:HD],
                            in_=mm[:].rearrange("p (h d) -> p h d", h=8),
                        )

            # ================= Stage B: attention =================
            with tc.tile_pool(name="ppool", bufs=3) as ppool, \
                 tc.tile_pool(name="ystage", bufs=5) as ystage, \
                 tc.tile_pool(name="rbp", bufs=2) as rbp, \
                 tc.tile_pool(name="ps_sg", bufs=3, space="PSUM") as ps_sg, \
                 tc.tile_pool(name="ps_av", bufs=2, space="PSUM") as ps_av:
                # B1 runs qt-outer so each q-tile's denominators finish early;
                # B3+C for q-tile qt are emitted after B1(qt+1)'s units, which
                # keeps the in-order PE queue busy while the reciprocal chain
                # runs on the DVE.  AV matmuls are emitted two exp-groups late
                # (software pipelining across unit boundaries) so the PE
                # always has scores work while ACT evaluates exps.
                from collections import deque
                pending = deque()

                def flush_one():
                    if pending:
                        pending.popleft()()

                def emit_unit(h, qt):
                    pr, half = h // 2, h % 2
                    q0 = qt * 512
                    nk = 4 * qt + 4
                    av = ps_av.tile([P, 512], f32, tag="avy", name=f"av{h}_{qt}")
                    for g0 in range(0, nk, EG):
                        gsz = min(EG, nk - g0)
                        sg = ps_sg.tile([P, EG * 512], f32, tag="sg",
                                        name=f"sg{h}_{qt}_{g0}")
                        for j in range(gsz):
                            kc = g0 + j
                            js = slice(j * 512, (j + 1) * 512)
                            nc.tensor.matmul(
                                sg[:, js],
                                kt_all[:, pr, kc * P : (kc + 1) * P],
                                q_all[:, h, q0 : q0 + 512],
                                start=True,
                                stop=True,
                            )
                            m = kc - 4 * qt
                            if m >= 0:
                                # triangular mask on the [128,128] diagonal
                                nc.vector.tensor_tensor(
                                    sg[:, j * 512 + m * P : j * 512 + (m + 1) * P],
                                    sg[:, j * 512 + m * P : j * 512 + (m + 1) * P],
                                    cm_sb[:],
                                    OP.add,
                                )
                        pt = ppool.tile([P, EG * 512], BF16, tag="pt",
                                        name=f"pt{h}_{qt}_{g0}")
                        nc.scalar.activation(
                            out=pt[:, 0 : gsz * 512],
                            in_=sg[:, 0 : gsz * 512],
                            func=AF.Exp,
                            scale=0.125,
                        )
                        if len(pending) >= 2:
                            flush_one()

                        def av_group(h=h, qt=qt, g0=g0, gsz=gsz, pt=pt, av=av,
                                     last=(g0 + gsz == nk)):
                            nk_ = 4 * qt + 4
                            for j in range(gsz):
                                kc = g0 + j
                                m = kc - 4 * qt
                                c0 = m * P if m > 0 else 0
                                nc.tensor.matmul(
                                    av[0:VW, c0:512],
                                    v_all[:, kc, h, :],
                                    pt[:, j * 512 + c0 : (j + 1) * 512],
                                    start=(kc == 0),
                                    stop=(kc == nk_ - 1),
                                    skip_group_check=True,
                                )
                            if not last:
                                return
                            # park unnormalized output + denominator (row
                            # 64+h of av holds head h's denominator, other
                            # rows are zero, so accumulating the aligned
                            # [64:72] block is exact)
                            if h == 0:
                                nc.vector.tensor_copy(
                                    out=den[64:72, qt, :],
                                    in_=av[64:72, :],
                                )
                            else:
                                nc.vector.tensor_tensor(
                                    den[64:72, qt, :],
                                    den[64:72, qt, :],
                                    av[64:72, :],
                                    OP.add,
                                )
                            po_ = HD * (h % 2)
                            nc.vector.tensor_copy(
                                out=attn_t[po_ : po_ + HD, h // 2,
                                           qt * 512 : qt * 512 + 512],
                                in_=av[0:HD, :],
                            )

                        pending.append(av_group)

                def emit_recip(qt):
                    nc.vector.reciprocal(den[64:72, qt, :], den[64:72, qt, :])

                def emit_b3(qt):
                    q0 = qt * 512
                    denb = rbp.tile([72, 512], BF16, tag="denb", name=f"denb{qt}")
                    nc.scalar.activation(
                        out=denb[64:72, :],
                        in_=den[64:72, qt, :],
                        func=AF.Identity,
                    )
                    for h in range(8):
                        pr, half = h // 2, h % 2
                        po = HD * half
                        rb = ps_av.tile([P, 512], f32, tag="avy",
                                        name=f"rb{qt}_{h}")
                        nc.tensor.matmul(
                            rb[0:HD, :],
                            sel_sb[64:72, h, :],
                            denb[64:72, :],
                            start=True,
                            stop=True,
                        )
                        nc.vector.tensor_tensor(
                            attn_t[po : po + HD, pr, q0 : q0 + 512],
                            attn_t[po : po + HD, pr, q0 : q0 + 512],
                            rb[0:HD, :],
                            OP.mult,
                        )

                def emit_c(qt, half=None):
                    qcs = range(4 * qt, 4 * qt + 4)
                    if half == 0:
                        qcs = range(4 * qt, 4 * qt + 2)
                    elif half == 1:
                        qcs = range(4 * qt + 2, 4 * qt + 4)
                    for qc in qcs:
                        q0 = qc * P
                        yt = ystage.tile([P, D], f32, tag="yt", name=f"yt{qc}")
                        for nb in range(2):
                            yp = ps_av.tile([P, 512], f32, tag="avy",
                                            name=f"yp{qc}_{nb}")
                            for pc in range(4):
                                nc.tensor.matmul(
                                    yp[:],
                                    attn_t[:, pc, q0 : q0 + P],
                                    wo_sb[:, pc, nb * 512 : (nb + 1) * 512],
                                    start=(pc == 0),
                                    stop=False,
                                )
                            nc.tensor.matmul(
                                yp[:],
                                ones[0:1, :],
                                bo_sb[0:1, nb * 512 : (nb + 1) * 512],
                                start=False,
                                stop=True,
                            )
                            nc.scalar.activation(
                                out=yt[:, nb * 512 : (nb + 1) * 512],
                                in_=yp[:],
                                func=AF.Identity,
                            )
                        nc.sync.dma_start(out=y_d[q0 : q0 + P, :], in_=yt[:])

                # B3/C for q-tile qt-1 are spread INSIDE round qt so the
                # reciprocal chain always has PE work as cover and the kernel
                # tail holds only B3(3) + C(3); half of C(2) is held back to
                # cover the final reciprocal chain.
                for qt in range(NST):
                    for h in range(8):
                        emit_unit(h, qt)
                        if qt >= 1:
                            if h == 1:
                                emit_recip(qt - 1)
                            elif h == 3:
                                emit_b3(qt - 1)
                            elif h == 5:
                                emit_c(qt - 1, half=0)
                            elif h == 7 and qt < NST - 1:
                                emit_c(qt - 1, half=1)
                while pending:
                    flush_one()
                emit_recip(NST - 1)
                emit_c(NST - 2, half=1)
                emit_b3(NST - 1)
                emit_c(NST - 1)

    nc.finalize()
    return nc


_NC = None


def _get_nc():
    global _NC
    if _NC is None:
        _NC = build_program()
    return _NC


def _shard_inputs(x, causal_mask, padding_mask, W_qkv, b_qkv, W_out, b_out):
    bf16 = ml_dtypes.bfloat16
    x = np.ascontiguousarray(np.asarray(x, dtype=np.float32))
    W_qkv = np.asarray(W_qkv, dtype=np.float32)
    b_qkv = np.asarray(b_qkv, dtype=np.float32)
    W_out = np.asarray(W_out, dtype=np.float32)
    b_out = np.asarray(b_out, dtype=np.float32)
    causal_mask = np.asarray(causal_mask)
    padding_mask = np.asarray(padding_mask)

    assert not padding_mask.any(), "kernel assumes no padding"
    # additive triangle for the [128,128] diagonal block of scores^T[k, q]:
    # masked iff local k > local q
    cm = np.where(
        causal_mask[0:P, 0:P].T, np.float32(NEG), np.float32(0.0)
    ).astype(bf16)
    sel = np.repeat(np.eye(8, dtype=np.float32)[:, :, None], HD, axis=2).astype(bf16)

    in_maps = []
    for c in range(8):
        b, g = c // 2, c % 2
        cols = slice(g * 512, (g + 1) * 512)
        # [3, 1024, 512]: per-projection weight slices for this head group
        w3 = np.stack(
            [W_qkv[:, 1024 * i : 1024 * (i + 1)][:, cols] for i in range(3)]
        )
        wv = np.ascontiguousarray(
            w3[2].reshape(DC, P, 512).transpose(1, 0, 2).astype(bf16)
        )
        fp8 = ml_dtypes.float8_e4m3
        w8 = np.ascontiguousarray(
            w3[0:2].reshape(2, 4, 2, P, 512).transpose(0, 3, 1, 2, 4).astype(fp8)
        )
        xt8 = np.ascontiguousarray(
            x[b].T.reshape(4, 2, P, NST, 512).transpose(2, 3, 0, 1, 4).astype(fp8)
        )
        b3 = np.stack([b_qkv[1024 * i : 1024 * (i + 1)][cols] for i in range(3)])
        bq12 = np.ascontiguousarray(b3[0:2].reshape(8, P).T.astype(np.float32))
        bq12 = np.concatenate(
            [bq12, np.zeros((P, 4), np.float32)], axis=1
        )  # [128, 12]; V-bias columns unused
        xt = np.ascontiguousarray(
            x[b].T.reshape(DC, P, NST, 512).transpose(1, 2, 0, 3).astype(bf16)
        )
        wo = np.ascontiguousarray(
            W_out[g * 512 : (g + 1) * 512, :]
            .reshape(4, P, D)
            .transpose(1, 0, 2)
            .astype(bf16)
        )
        in_maps.append(
            {
                "xt": xt,
                "wv": wv,
                "w8qk": w8,
                "xt8": xt8,
                "bq12": bq12,
                "bv": np.ascontiguousarray(b3[2:3].astype(bf16)),
                "bo": (b_out if g == 0 else np.zeros_like(b_out))[None, :].astype(bf16),
                "wout": wo,
                "cmtri": cm,
                "sel": sel,
            }
        )
    return in_maps


def _run(in_maps, **kwargs):
    nc = _get_nc()
    return run_bass_kernel_spmd(nc, in_maps, core_ids=list(range(8)), **kwargs)


def kernel(**inputs):
    in_maps = _shard_inputs(**inputs)
    res = _run(in_maps)
    out = np.empty((B, S, D), dtype=np.float32)
    for b in range(B):
        out[b] = res.results[2 * b]["y"] + res.results[2 * b + 1]["y"]
    return out


def kernel_traced(**inputs):
    """Like kernel() but with NTFF tracing; returns (out, BassKernelResults)."""
    in_maps = _shard_inputs(**inputs)
    res = _run(in_maps, trace=True)
    out = np.empty((B, S, D), dtype=np.float32)
    for b in range(B):
        out[b] = res.results[2 * b]["y"] + res.results[2 * b + 1]["y"]
    return out, res
